# revision 5
# baseline (speedup 1.0000x reference)
"""Hard-mining JointsMSELoss on 8 Trainium2 NeuronCores — v4.

Math identical to v2/v3 (see kernel_v2.py): d = P-T (DVE), q = d^2 + ones
column (ACT), per-joint trace matmuls T_b^T @ [q_b | 1] (PE) giving
T-weighted masked SSE (diag) + sum-T (col 128), diag extracted by a fused
scalar_tensor_tensor, hard-negative max = max(d) via TT-max folds.

v4 over v3 (52.2us):
  - both joints of a wave share ONE single-bank PSUM tile ([128,2,129]),
    tag bufs=6: PE can run ~6 waves ahead of the DVE diag-extract, so PE
    never stalls on PSUM recycling through the busy DVE.
  - PE warm-up dropped (pair rate is LDWEIGHTS-bound at ~107ns; the HAM
    clock state doesn't matter).
  - max path batched over PAIRS of waves (4 joints per fold chain).
  - final output DMAs on the idle GpSimd queue.
"""

import os
import sys

sys.path.insert(0, "/opt/trn_rl_repo")

import ml_dtypes
import numpy as np

import concourse.bacc as bacc
import concourse.mybir as mybir
import concourse.tile as tile
from concourse.bass_utils import run_bass_kernel_spmd

B, J, H, W = 64, 17, 128, 128
NCORES = 8
BL = B // NCORES          # local batch per core
FD = BL * W               # free dim per joint tile (1024)
NB = FD // 128            # 128-col blocks per joint (8)
WAVE = 2                  # joints per DMA/compute wave

BF16 = ml_dtypes.bfloat16

_CACHE = {}


def _build():
    f32 = mybir.dt.float32
    bf16 = mybir.dt.bfloat16
    Alu = mybir.AluOpType
    Act = mybir.ActivationFunctionType
    nc = bacc.Bacc(
        "TRN2",
        target_bir_lowering=False,
        debug=False,
        enable_asserts=False,
    )
    P_d = nc.dram_tensor("out_x", [J, H, BL, W], bf16, kind="ExternalInput")
    T_d = nc.dram_tensor("tgt_x", [J, H, BL, W], bf16, kind="ExternalInput")
    s_d = nc.dram_tensor("s_col", [H, J], f32, kind="ExternalOutput")
    c_d = nc.dram_tensor("c_col", [H, J], f32, kind="ExternalOutput")
    m_d = nc.dram_tensor("mx_col", [H, J], f32, kind="ExternalOutput")

    P_re = P_d.ap().rearrange("j h b w -> h j (b w)")   # [H, J, FD]
    T_re = T_d.ap().rearrange("j h b w -> h j (b w)")

    Ddiag_d = nc.inline_tensor(np.eye(H, dtype=np.float32).astype(BF16), name="ddiag")

    waves = [(j0, min(WAVE, J - j0)) for j0 in range(0, J, WAVE)]

    with tile.TileContext(nc) as tc:
        with (
            tc.tile_pool(name="io", bufs=3) as io,
            tc.tile_pool(name="work", bufs=3) as work,
            tc.tile_pool(name="psum", bufs=8, space="PSUM") as psum,
            tc.tile_pool(name="const", bufs=1) as const,
            tc.tile_pool(name="acc", bufs=1) as accp,
        ):
            Dg = const.tile([H, H], bf16, tag="ddiag")
            nc.sync.dma_start(out=Dg[:], in_=Ddiag_d.ap())
            ones = const.tile([H, WAVE * NB], bf16, tag="ones")
            nc.vector.memset(ones[:], 1.0)
            s_col = accp.tile([H, J], f32, tag="s")
            c_col = accp.tile([H, J], f32, tag="c")
            mx_col = accp.tile([H, J], f32, tag="mx")

            prev_max = None  # deferred (pair-level) max-path emission

            def emit_maxpath(args):
                j0, npj, d = args
                t1 = work.tile([H, 2 * WAVE, 512], bf16, tag="t1")
                nc.vector.tensor_tensor(
                    t1[:, :npj], d[:, :npj, 0:512], d[:, :npj, 512:1024], Alu.max
                )
                t2 = work.tile([H, 2 * WAVE, 256], bf16, tag="t2")
                nc.vector.tensor_tensor(
                    t2[:, :npj], t1[:, :npj, 0:256], t1[:, :npj, 256:512], Alu.max
                )
                nc.vector.reduce_max(
                    mx_col[:, j0 : j0 + npj], t2[:, :npj], axis=mybir.AxisListType.X
                )

            d = None
            for wi, (j0, nj) in enumerate(waves):
                half = wi % 2
                if half == 0:
                    d = work.tile([H, 2 * WAVE, FD], bf16, tag="d", bufs=5)
                    pair_j0 = j0
                    pair_n = 0

                Pt = io.tile([H, WAVE, FD], bf16, tag="P", bufs=8)
                Tt = io.tile([H, WAVE, FD], bf16, tag="T", bufs=9)
                nc.sync.dma_start(out=Pt[:, :nj], in_=P_re[:, j0 : j0 + nj])
                nc.sync.dma_start(out=Tt[:, :nj], in_=T_re[:, j0 : j0 + nj])

                # d = P - T  (bf16 SBUF, 2x mode)
                dsl = d[:, half * WAVE : half * WAVE + nj]
                nc.vector.tensor_sub(dsl, Pt[:, :nj], Tt[:, :nj])
                pair_n += nj

                # emit previous pair's max path after the first sub of a new
                # pair so subtracts stay at the head of the DVE queue
                if half == 0 and prev_max is not None:
                    emit_maxpath(prev_max)
                    prev_max = None

                # q = d^2 into [nj, NB, 129] layout; col 128 of each block = 1
                q = work.tile([H, WAVE, NB, 129], bf16, tag="q", bufs=6)
                nc.scalar.activation(
                    q[:, :nj, :, 128:129],
                    ones[:, : nj * NB].rearrange("h (j nb c) -> h j nb c", j=nj, c=1),
                    Act.Copy,
                )
                nc.scalar.activation(
                    q[:, :nj, :, :128],
                    dsl.rearrange("h j (nb c) -> h j nb c", nb=NB),
                    Act.Square,
                )

                # PE trace: M_k = sum_b T_b^T @ [q_b | 1]; both joints of the
                # wave share one single-bank PSUM tile
                Mps = psum.tile([H, WAVE, 129], f32, tag="M")
                for k in range(nj):
                    j = j0 + k
                    for b in range(NB):
                        nc.tensor.matmul(
                            Mps[:, k],
                            Tt[:, k, b * 128 : (b + 1) * 128],
                            q[:, k, b],
                            start=(b == 0),
                            stop=(b == NB - 1),
                        )
                for k in range(nj):
                    j = j0 + k
                    scr = work.tile([H, H], f32, tag="scr")
                    nc.vector.scalar_tensor_tensor(
                        scr[:],
                        Mps[:, k, :128],
                        0.0,
                        Dg[:],
                        Alu.bypass,
                        Alu.mult,
                        accum_out=s_col[:, j : j + 1],
                    )
                    nc.scalar.activation(
                        c_col[:, j : j + 1], Mps[:, k, 128:129], Act.Copy
                    )

                if half == 1 or wi == len(waves) - 1:
                    prev_max = (pair_j0, pair_n, d)

            emit_maxpath(prev_max)

            nc.gpsimd.dma_start(out=s_d.ap(), in_=s_col[:])
            nc.gpsimd.dma_start(out=c_d.ap(), in_=c_col[:])
            nc.gpsimd.dma_start(out=m_d.ap(), in_=mx_col[:])
    nc.compile()
    return nc


def run(output, target, trace=False, tmpdir=None):
    """Returns (loss, BassKernelResults)."""
    if "nc" not in _CACHE:
        _CACHE["nc"] = _build()
    nc = _CACHE["nc"]

    output = np.asarray(output)
    target = np.asarray(target)
    in_maps = []
    for c in range(NCORES):
        sl = slice(c * BL, (c + 1) * BL)
        in_maps.append(
            {
                "out_x": np.ascontiguousarray(
                    output[sl].transpose(1, 2, 0, 3)
                ).astype(BF16),
                "tgt_x": np.ascontiguousarray(
                    target[sl].transpose(1, 2, 0, 3)
                ).astype(BF16),
            }
        )
    res = run_bass_kernel_spmd(
        nc, in_maps, list(range(NCORES)), trace=trace, tmpdir=tmpdir
    )

    s = np.zeros(J, np.float64)
    c = np.zeros(J, np.float64)
    mx = np.full(J, -np.inf)
    for r in res.results:
        s += r["s_col"].astype(np.float64).sum(axis=0)
        c += r["c_col"].astype(np.float64).sum(axis=0)
        mx = np.maximum(mx, r["mx_col"].max(axis=0))
    loss = np.float32((s / c + mx * mx).mean())
    return loss, res


def kernel(output, target):
    return run(output, target, trace=os.environ.get("BASS_KERNEL_TRACE") == "1")[0]
